# revision 1
# baseline (speedup 1.0000x reference)
"""AIMNet2 interaction module on 8 TRN2 NeuronCores.

Strategy
--------
Algebraic restructure: the nn.Linear commutes with the segment-sum, so we
accumulate A[n, ch, f] = sum_{p: idx_i[p]=n} c_ch[p] * E[idx_j[p], f] with
c = f_ij * [1, ux, uy, uz] (4 channels), then apply W on the [N,3,F] result
plus a count_n * b correction, then the norm.  This cuts matmul work 16x and
avoids materializing [P,3,F].

Sharding: pairs sorted by idx_i on host; each core owns a contiguous range of
2500 target atoms and all pairs whose idx_i lands in it -> zero inter-core
communication.  Atoms are greedy-packed into windows (<=32 atoms AND <=512
pairs each; ~84 windows/core, adaptive).  Each core returns its transposed
output slice [256, N_LOC]; host concatenates, unpermutes, transposes.

Per-core device pipeline (raw bass, manual semaphores; phase 2 interleaved
into the window loop of every engine so only the last group is kernel tail):
  gpsimd : 42x dma_gather (1024 rows x 256B bf16) of neighbor embeddings
           (desc-gen bound ~10ns/row -> this stream IS the kernel runtime)
  DVE    : per-window "weighted one-hot" rhs [128p, (4ch x 32a)] via
           iota/is_equal + broadcast-mult; per-group sum-of-squares adds
  PE     : per 128-pair chunk: psum[f, (ch,a)] += a_j^T @ wone (4 chunks per
           window accumulate in one psum tile); per 4-window group the
           W-transform psum2[g, (w,c,a)] = wt.T @ vec + b x counts
  ACT    : psum evacuations (vec->bf16, radial->f32), Square, Sqrt
  sync   : all input DMAs + per-group streamed output DMAs of [256, N_LOC]
"""
import sys, types
import numpy as np
import ml_dtypes

sys.path.insert(0, "/opt/trn_rl_repo")

import concourse.bass as bass
import concourse.bacc as bacc
import concourse.mybir as mybir
from concourse.bass_utils import run_bass_kernel_spmd
from concourse.library_config import mlp

# ---------------- problem constants (hardcoded per spec) ----------------
N_ATOMS = 20000
F = 128
N_CORES = 8
ATOMS_PER_CORE = 2500          # 8 * 2500 = 20000
WIN = 32                       # max atoms per window (psum col budget)
N_WIN = 84                     # windows per core (greedy-packed; adaptive)
N_LOC = N_WIN * WIN            # padded local atom slots
K_CH = 4                       # chunks (x128 pairs) per window -> 512 slots
SLOTS_PER_WIN = K_CH * 128
N_SLOT = N_WIN * SLOTS_PER_WIN  # 51200
N_CHUNK = N_SLOT // 128        # 400
GCH = 1024                     # rows per dma_gather call (ring-overflow safe)
N_GATHER = N_SLOT // GCH       # 50
G_DEPTH = 16                   # outstanding gathers
NPSUM1 = 6                     # live phase-1 psum tiles
NWONE = 8                      # live wone tiles
PH2G = 2                       # windows per phase-2 group
N_PH2 = N_WIN // PH2G
EPS = 1e-12

bf16 = mybir.dt.bfloat16
f32 = mybir.dt.float32
i16 = mybir.dt.int16

_cache = {}


def _build_graph():
    nc = bacc.Bacc("TRN2", debug=False)
    dp = nc.declare_dram_parameter
    table = dp("table", [N_ATOMS, F], bf16, isOutput=False)
    gidx = dp("gidx", [128, N_SLOT // 16], i16, isOutput=False)
    a32 = dp("a32", [128, N_CHUNK], bf16, isOutput=False)
    coef = dp("coef", [128, N_CHUNK, 4], bf16, isOutput=False)
    iota = dp("iota", [128, K_CH, WIN], bf16, isOutput=False)
    wt = dp("wt", [F, F], bf16, isOutput=False)          # W transposed: wt[f,g]=W[g,f]
    bvec = dp("bvec", [1, F], bf16, isOutput=False)
    cnt3 = dp("cnt3", [1, N_WIN * 96], bf16, isOutput=False)
    wcnt = dp("wcnt", [1, N_WIN], mybir.dt.int32, isOutput=False)
    out = dp("out", [256, N_LOC], f32, isOutput=True)

    import contextlib
    with contextlib.ExitStack() as ctx:
        E = ctx.enter_context
        block = E(nc.Block())
        gath = E(nc.sbuf_tensor("gath", [128, N_CHUNK, F], bf16))
        gidx_sb = E(nc.sbuf_tensor("gidx_sb", [128, N_SLOT // 16], i16))
        a32_sb = E(nc.sbuf_tensor("a32_sb", [128, N_CHUNK], bf16))
        coef_sb = E(nc.sbuf_tensor("coef_sb", [128, N_CHUNK, 4], bf16))
        iota_sb = E(nc.sbuf_tensor("iota_sb", [128, K_CH, WIN], bf16))
        wt_sb = E(nc.sbuf_tensor("wt_sb", [F, F], bf16))
        bvec_sb = E(nc.sbuf_tensor("bvec_sb", [1, F], bf16))
        cnt3_sb = E(nc.sbuf_tensor("cnt3_sb", [1, N_WIN * 96], bf16))
        wcnt_sb = E(nc.sbuf_tensor("wcnt_sb", [1, N_WIN], mybir.dt.int32))
        eq_sb = E(nc.sbuf_tensor("eq_sb", [128, K_CH, WIN], bf16))
        wone_sb = E(nc.sbuf_tensor("wone_sb", [128, NWONE, K_CH, 4 * WIN], bf16))
        vec_sb = E(nc.sbuf_tensor("vec_sb", [128, N_WIN * 96], bf16))
        rad_sb = E(nc.sbuf_tensor("rad_sb", [128, N_LOC], f32))
        vnorm_sb = E(nc.sbuf_tensor("vnorm_sb", [128, N_LOC], f32))
        sq_sb = E(nc.sbuf_tensor("sq_sb", [128, 2, PH2G * 96], f32))
        vsq_sb = E(nc.sbuf_tensor("vsq_sb", [128, 2, PH2G * WIN], f32))
        eps_sb = E(nc.sbuf_tensor("eps_sb", [128, 1], f32))
        psum1 = [E(nc.psum_tensor(f"ps1_{i}", [128, 128], f32)) for i in range(NPSUM1)]
        psum2 = [E(nc.psum_tensor(f"ps2_{i}", [128, PH2G * 96], f32)) for i in range(2)]

        io = E(nc.semaphore("io"))
        gisem = E(nc.semaphore("gisem"))
        gsem = E(nc.semaphore("gsem"))
        dve_sem = E(nc.semaphore("dve_sem"))
        pe_win = E(nc.semaphore("pe_win"))
        evac = E(nc.semaphore("evac"))
        pe2 = E(nc.semaphore("pe2"))
        sqs = E(nc.semaphore("sqs"))
        vsqs = E(nc.semaphore("vsqs"))
        vns = E(nc.semaphore("vns"))
        outs = E(nc.semaphore("outs"))

        N_IN_DMAS = 7
        IO_READY = 16 * N_IN_DMAS

        @block.gpsimd
        def _(g: bass.BassGpSimd):
            g.load_library(mlp)
            g.wait_ge(gisem, 16)
            cpg = GCH // 128   # sbuf chunks per gather
            ipg = GCH // 16    # idx cols per gather
            for s in range(N_GATHER):
                if s >= G_DEPTH:
                    g.wait_ge(gsem, 16 * (s - G_DEPTH + 1))
                g.dma_gather(
                    gath[:, s * cpg:(s + 1) * cpg, :],
                    table[:],
                    gidx_sb[:, s * ipg:(s + 1) * ipg],
                    GCH, GCH, F,
                ).then_inc(gsem, 16)

        @block.vector
        def _(v: bass.BassVectorEngine):
            v.memset(eps_sb[:], EPS)
            v.wait_ge(io, IO_READY)
            for w in range(N_WIN):
                if w >= NWONE:
                    v.wait_ge(pe_win, w - NWONE + 1)
                # eq[p, k, a] = (a32[p, w*K_CH+k] == iota[a])
                v.tensor_tensor(
                    out=eq_sb[:],
                    in0=a32_sb[:, K_CH * w:K_CH * (w + 1), None].to_broadcast(
                        [128, K_CH, WIN]),
                    in1=iota_sb[:],
                    op=mybir.AluOpType.is_equal,
                )
                # wone[p, k, ch, a] = eq[p, k, a] * coef[p, 5w+k, ch]
                v.tensor_tensor(
                    out=wone_sb[:, w % NWONE].rearrange("p k (c a) -> p k c a", c=4),
                    in0=eq_sb[:, :, None, :].to_broadcast([128, K_CH, 4, WIN]),
                    in1=coef_sb[:, K_CH * w:K_CH * (w + 1), :, None].to_broadcast(
                        [128, K_CH, 4, WIN]),
                    op=mybir.AluOpType.mult,
                ).then_inc(dve_sem, 1)
                if w % PH2G == PH2G - 1:
                    gi = w // PH2G
                    if gi >= 2:
                        v.wait_ge(vns, gi - 1)
                    v.wait_ge(sqs, gi + 1)
                    s3 = sq_sb[:, gi % 2].rearrange("p (w c a) -> p w c a", c=3, a=WIN)
                    v.tensor_tensor(
                        out=vsq_sb[:, gi % 2].rearrange("p (w a) -> p w a", a=WIN),
                        in0=s3[:, :, 0, :],
                        in1=s3[:, :, 1, :],
                        op=mybir.AluOpType.add,
                    )
                    v.tensor_tensor(
                        out=vsq_sb[:, gi % 2].rearrange("p (w a) -> p w a", a=WIN),
                        in0=vsq_sb[:, gi % 2].rearrange("p (w a) -> p w a", a=WIN),
                        in1=s3[:, :, 2, :],
                        op=mybir.AluOpType.add,
                    ).then_inc(vsqs, 1)

        @block.tensor
        def _(t: bass.BassTensorEngine):
            for w in range(N_WIN):
                ps = psum1[w % NPSUM1]
                t.wait_ge(dve_sem, w + 1)
                last_g = (K_CH * w + K_CH - 1) // (GCH // 128)
                t.wait_ge(gsem, 16 * (last_g + 1))
                if w >= NPSUM1:
                    t.wait_ge(evac, 2 * (w - NPSUM1 + 1))
                for k in range(K_CH):
                    mm = t.matmul(
                        out=ps[:],
                        lhsT=gath[:, K_CH * w + k, :],
                        rhs=wone_sb[:, w % NWONE, k, :],
                        start=(k == 0),
                        stop=(k == K_CH - 1),
                    )
                    if k == K_CH - 1:
                        mm.then_inc(pe_win, 1)
                if w % PH2G == PH2G - 1:
                    gi = w // PH2G
                    t.wait_ge(evac, 2 * PH2G * (gi + 1))
                    if gi >= 2:
                        t.wait_ge(sqs, gi - 1)
                    t.matmul(
                        out=psum2[gi % 2][:],
                        lhsT=wt_sb[:],
                        rhs=vec_sb[:, gi * PH2G * 96:(gi + 1) * PH2G * 96],
                        start=True, stop=False,
                    )
                    t.matmul(
                        out=psum2[gi % 2][:],
                        lhsT=bvec_sb[:],
                        rhs=cnt3_sb[:, gi * PH2G * 96:(gi + 1) * PH2G * 96],
                        start=False, stop=True,
                    ).then_inc(pe2, 1)

        @block.scalar
        def _(a: bass.BassEngine):
            Copy = mybir.ActivationFunctionType.Copy
            for w in range(N_WIN):
                a.wait_ge(pe_win, w + 1)
                ps = psum1[w % NPSUM1]
                a.activation(out=vec_sb[:, w * 96:(w + 1) * 96],
                             in_=ps[:, WIN:128], func=Copy).then_inc(evac, 1)
                a.activation(out=rad_sb[:, w * WIN:(w + 1) * WIN],
                             in_=ps[:, 0:WIN], func=Copy).then_inc(evac, 1)
                if w % PH2G == PH2G - 1:
                    gi = w // PH2G
                    a.wait_ge(pe2, gi + 1)
                    if gi >= 2:
                        a.wait_ge(vsqs, gi - 1)
                    a.activation(out=sq_sb[:, gi % 2], in_=psum2[gi % 2][:],
                                 func=mybir.ActivationFunctionType.Square,
                                 ).then_inc(sqs, 1)
                    a.wait_ge(vsqs, gi + 1)
                    a.activation(out=vnorm_sb[:, gi * PH2G * WIN:(gi + 1) * PH2G * WIN],
                                 in_=vsq_sb[:, gi % 2],
                                 func=mybir.ActivationFunctionType.Sqrt,
                                 bias=eps_sb[:, 0:1]).then_inc(vns, 1)

        @block.sync
        def _(s: bass.BassEngine):
            s.dma_start(gidx_sb[:], gidx[:]).then_inc(gisem, 16)
            s.dma_start(a32_sb[:], a32[:]).then_inc(io, 16)
            s.dma_start(coef_sb[:], coef[:]).then_inc(io, 16)
            s.dma_start(iota_sb[:], iota[:]).then_inc(io, 16)
            s.dma_start(wt_sb[:], wt[:]).then_inc(io, 16)
            s.dma_start(bvec_sb[:], bvec[:]).then_inc(io, 16)
            s.dma_start(cnt3_sb[:], cnt3[:]).then_inc(io, 16)
            s.dma_start(wcnt_sb[:], wcnt[:]).then_inc(io, 16)
            for gi in range(N_PH2):
                c0, c1 = gi * PH2G * WIN, (gi + 1) * PH2G * WIN
                s.wait_ge(evac, 2 * PH2G * (gi + 1))
                s.dma_start(out[128:256, c0:c1], rad_sb[:, c0:c1]).then_inc(outs, 16)
                s.wait_ge(vns, gi + 1)
                s.dma_start(out[0:128, c0:c1], vnorm_sb[:, c0:c1]).then_inc(outs, 16)
            s.wait_ge(outs, 32 * N_PH2)

    nc.compile()
    return nc


def _prep_core(idx_i, idx_j, coef4, base):
    """Build per-core host arrays. idx_* already filtered+sorted by idx_i.

    Greedy variable-atom windows: consecutive local atoms are packed into a
    window until it would exceed SLOTS_PER_WIN pairs or WIN atoms."""
    a_loc = idx_i - base                       # [p] in [0, ATOMS_PER_CORE)
    counts = np.bincount(a_loc, minlength=ATOMS_PER_CORE)
    atom_win = np.zeros(ATOMS_PER_CORE, dtype=np.int64)
    atom_rank = np.zeros(ATOMS_PER_CORE, dtype=np.int64)
    w = acc = na = 0
    for atom in range(ATOMS_PER_CORE):
        c = int(counts[atom])
        if acc + c > SLOTS_PER_WIN or na == WIN:
            w += 1
            acc = na = 0
        atom_win[atom] = w
        atom_rank[atom] = na
        acc += c
        na += 1
    if w >= N_WIN:
        raise RuntimeError(f"needs {w + 1} windows > {N_WIN}")
    win = atom_win[a_loc]
    jidx = np.zeros(N_SLOT, dtype=np.int16)
    a32v = np.zeros(N_SLOT, dtype=np.float32)
    cf = np.zeros((N_SLOT, 4), dtype=np.float32)
    cnt_w = np.bincount(win, minlength=N_WIN)
    # pairs are sorted by idx_i hence grouped by window
    starts_in = np.concatenate([[0], np.cumsum(cnt_w)[:-1]])
    for wi in range(N_WIN):
        n = cnt_w[wi]
        if n == 0:
            continue
        s0, d0 = starts_in[wi], wi * SLOTS_PER_WIN
        jidx[d0:d0 + n] = idx_j[s0:s0 + n]
        a32v[d0:d0 + n] = atom_rank[a_loc[s0:s0 + n]].astype(np.float32)
        cf[d0:d0 + n] = coef4[s0:s0 + n]
    # gather idx wrap: per window-call, [16, GCH//16] blocks
    gidx_h = np.tile(
        jidx.reshape(N_GATHER, GCH // 16, 16).transpose(2, 0, 1).reshape(16, -1),
        (8, 1))
    wcnt_h = cnt_w.astype(np.int32).reshape(1, -1)
    # slot -> (partition, chunk): slot = chunk*128 + p
    a32_h = a32v.reshape(N_CHUNK, 128).T.astype(ml_dtypes.bfloat16)
    coef_h = np.ascontiguousarray(
        cf.reshape(N_CHUNK, 128, 4).transpose(1, 0, 2)).astype(ml_dtypes.bfloat16)
    # counts replicated over 3 vec channels: [w, c, a-rank]
    col_of = (atom_win * WIN + atom_rank).astype(np.int64)
    cnts_col = np.zeros(N_LOC, dtype=np.float32)
    cnts_col[col_of] = counts
    cnt3_h = np.broadcast_to(
        cnts_col.reshape(N_WIN, 1, WIN), (N_WIN, 3, WIN)).reshape(1, -1)
    return (gidx_h, a32_h, coef_h,
            np.ascontiguousarray(cnt3_h).astype(ml_dtypes.bfloat16), wcnt_h, col_of)


def _windows_needed(a_loc):
    counts = np.bincount(a_loc, minlength=ATOMS_PER_CORE)
    w = acc = na = 0
    for atom in range(ATOMS_PER_CORE):
        c = int(counts[atom])
        if acc + c > SLOTS_PER_WIN or na == WIN:
            w += 1
            acc = na = 0
        acc += c
        na += 1
    return w + 1


def _set_n_win(nw):
    g = globals()
    g["N_WIN"] = nw
    g["N_LOC"] = nw * WIN
    g["N_SLOT"] = nw * SLOTS_PER_WIN
    g["N_CHUNK"] = g["N_SLOT"] // 128
    g["N_GATHER"] = g["N_SLOT"] // GCH
    g["N_PH2"] = nw // PH2G


def kernel(atomic_embedding, pairlist, f_ij_cutoff, r_ij, W, b):
    atomic_embedding = np.asarray(atomic_embedding, dtype=np.float32)
    pairlist = np.asarray(pairlist)
    f_ij = np.asarray(f_ij_cutoff, dtype=np.float32).reshape(-1)
    r_ij = np.asarray(r_ij, dtype=np.float32)
    W = np.asarray(W, dtype=np.float32)
    b = np.asarray(b, dtype=np.float32)

    u = r_ij / np.linalg.norm(r_ij, axis=1, keepdims=True)
    coef4 = np.concatenate([f_ij[:, None], f_ij[:, None] * u], axis=1)  # [P,4]

    idx_i = np.asarray(pairlist[0], dtype=np.int64)
    idx_j = np.asarray(pairlist[1], dtype=np.int64)
    order = np.argsort(idx_i, kind="stable")
    idx_i_s, idx_j_s, coef_s = idx_i[order], idx_j[order], coef4[order]

    table = atomic_embedding.astype(ml_dtypes.bfloat16)
    iota_h = np.broadcast_to(
        np.arange(WIN, dtype=np.float32), (128, K_CH, WIN))
    iota_h = np.ascontiguousarray(iota_h).astype(ml_dtypes.bfloat16)
    wt_h = np.ascontiguousarray(W.T).astype(ml_dtypes.bfloat16)
    b_h = b.reshape(1, F).astype(ml_dtypes.bfloat16)

    bounds = np.searchsorted(idx_i_s, np.arange(0, N_ATOMS + 1, ATOMS_PER_CORE))
    need = max(_windows_needed(idx_i_s[bounds[c]:bounds[c + 1]] - c * ATOMS_PER_CORE)
               for c in range(N_CORES))
    # round up: multiple of PH2G (phase-2 groups) and of 2 (1024-idx gathers)
    nw = -(-max(need, 16) // 2) * 2
    if nw != N_WIN:
        _cache.pop("nc", None)
    _set_n_win(nw)
    in_maps = []
    colmaps = []
    for c in range(N_CORES):
        lo, hi = bounds[c], bounds[c + 1]
        gidx_h, a32_h, coef_h, cnt3_h, wcnt_h, col_of = _prep_core(
            idx_i_s[lo:hi], idx_j_s[lo:hi], coef_s[lo:hi], c * ATOMS_PER_CORE)
        in_maps.append({
            "table": table, "gidx": gidx_h, "a32": a32_h, "coef": coef_h,
            "iota": iota_h, "wt": wt_h, "bvec": b_h, "cnt3": cnt3_h,
            "wcnt": wcnt_h,
        })
        colmaps.append(col_of)

    if "nc" not in _cache:
        _cache["nc"] = _build_graph()
    res = run_bass_kernel_spmd(_cache["nc"], in_maps, core_ids=list(range(N_CORES)))

    out_full = np.empty((N_ATOMS, 2 * F), dtype=np.float32)
    for c in range(N_CORES):
        o = res.results[c]["out"]  # [256, N_LOC]
        n = ATOMS_PER_CORE
        out_full[c * n:(c + 1) * n, :] = o[:, colmaps[c]].T
    return out_full



# revision 6
# speedup vs baseline: 2.2091x; 2.2091x over previous
"""AIMNet2 interaction module on 8 TRN2 NeuronCores.

Strategy
--------
Algebraic restructure: the nn.Linear commutes with the segment-sum, so we
accumulate A[n, ch, f] = sum_{p: idx_i[p]=n} c_ch[p] * E[idx_j[p], f] with
c = f_ij * [1, ux, uy, uz] (4 channels), then apply W on the [N,3,F] result
plus a count_n * b correction, then the norm.  This cuts matmul work 16x and
avoids materializing [P,3,F].

Sharding: pairs sorted by idx_i on host; each core owns a contiguous range of
2500 target atoms and all pairs whose idx_i lands in it -> zero inter-core
communication.  Atoms are greedy-packed into windows (<=32 atoms AND <=512
pairs each; ~84 windows/core, adaptive).  Each core returns its transposed
output slice [256, N_LOC]; host concatenates, unpermutes, transposes.

Per-core device pipeline (raw bass, manual semaphores; phase 2 interleaved
into the window loop of every engine so only the last group is kernel tail):
  gpsimd : 42x dma_gather (1024 rows x 256B bf16) of neighbor embeddings
           (desc-gen bound ~10ns/row -> this stream IS the kernel runtime)
  DVE    : per-window "weighted one-hot" rhs [128p, (4ch x 32a)] via
           iota/is_equal + broadcast-mult; per-group sum-of-squares adds
  PE     : per 128-pair chunk: psum[f, (ch,a)] += a_j^T @ wone (4 chunks per
           window accumulate in one psum tile); per 4-window group the
           W-transform psum2[g, (w,c,a)] = wt.T @ vec + b x counts
  ACT    : psum evacuations (vec->bf16, radial->f32), Square, Sqrt
  sync   : all input DMAs + per-group streamed output DMAs of [256, N_LOC]
"""
import sys, types
import numpy as np
import ml_dtypes

sys.path.insert(0, "/opt/trn_rl_repo")

import concourse.bass as bass
import concourse.bacc as bacc
import concourse.mybir as mybir
from concourse.bass_utils import run_bass_kernel_spmd
from concourse.library_config import mlp

# ---------------- problem constants (hardcoded per spec) ----------------
N_ATOMS = 20000
F = 128
N_CORES = 8
ATOMS_PER_CORE = 2500          # 8 * 2500 = 20000
WIN = 32                       # max atoms per window (psum col budget)
N_WIN = 84                     # windows per core (greedy-packed; adaptive)
N_LOC = N_WIN * WIN            # padded local atom slots
K_CH = 4                       # chunks (x128 pairs) per window -> 512 slots
SLOTS_PER_WIN = K_CH * 128
N_SLOT = N_WIN * SLOTS_PER_WIN  # 51200
N_CHUNK = N_SLOT // 128        # 400
GCH = 1024                     # rows per dma_gather call (ring-overflow safe)
N_GATHER = N_SLOT // GCH       # 50
G_DEPTH = 16                   # outstanding gathers
NPSUM1 = 6                     # live phase-1 psum tiles
NWONE = 8                      # live wone tiles
PH2G = 2                       # windows per phase-2 group
N_PH2 = N_WIN // PH2G
EPS = 1e-12

bf16 = mybir.dt.bfloat16
f32 = mybir.dt.float32
i16 = mybir.dt.int16

_cache = {}


N_Q = 4          # SWDGE queues (desc-gen cpu pairs) used round-robin
QDEPTH = 4       # outstanding gathers per queue (16 total, ring-safe)


def _gq(s):
    return s % N_Q


def _qcnt(s, q):
    """#gathers with index <= s on queue q."""
    if s < q:
        return 0
    return (s - q) // N_Q + 1


def _build_graph():
    nc = bacc.Bacc("TRN2", debug=False, num_swdge_queues=N_Q)
    dp = nc.declare_dram_parameter
    table = dp("table", [N_ATOMS, F], bf16, isOutput=False)
    gidx = dp("gidx", [128, N_SLOT // 16], i16, isOutput=False)
    a32 = dp("a32", [128, N_CHUNK], bf16, isOutput=False)
    coef = dp("coef", [128, N_CHUNK, 4], bf16, isOutput=False)
    iota = dp("iota", [128, K_CH, WIN], bf16, isOutput=False)
    wt = dp("wt", [F, F], bf16, isOutput=False)          # W transposed: wt[f,g]=W[g,f]
    bvec = dp("bvec", [1, F], bf16, isOutput=False)
    cnt3 = dp("cnt3", [1, N_WIN * 96], bf16, isOutput=False)
    wcnt = dp("wcnt", [1, N_WIN], mybir.dt.int32, isOutput=False)
    out = dp("out", [256, N_LOC], f32, isOutput=True)

    import contextlib
    with contextlib.ExitStack() as ctx:
        E = ctx.enter_context
        block = E(nc.Block())
        gath = E(nc.sbuf_tensor("gath", [128, N_CHUNK, F], bf16))
        gidx_sb = E(nc.sbuf_tensor("gidx_sb", [128, N_SLOT // 16], i16))
        a32_sb = E(nc.sbuf_tensor("a32_sb", [128, N_CHUNK], bf16))
        coef_sb = E(nc.sbuf_tensor("coef_sb", [128, N_CHUNK, 4], bf16))
        iota_sb = E(nc.sbuf_tensor("iota_sb", [128, K_CH, WIN], bf16))
        wt_sb = E(nc.sbuf_tensor("wt_sb", [F, F], bf16))
        bvec_sb = E(nc.sbuf_tensor("bvec_sb", [1, F], bf16))
        cnt3_sb = E(nc.sbuf_tensor("cnt3_sb", [1, N_WIN * 96], bf16))
        wcnt_sb = E(nc.sbuf_tensor("wcnt_sb", [1, N_WIN], mybir.dt.int32))
        eq_sb = E(nc.sbuf_tensor("eq_sb", [128, K_CH, WIN], bf16))
        wone_sb = E(nc.sbuf_tensor("wone_sb", [128, NWONE, K_CH, 4 * WIN], bf16))
        vec_sb = E(nc.sbuf_tensor("vec_sb", [128, N_WIN * 96], bf16))
        rad_sb = E(nc.sbuf_tensor("rad_sb", [128, N_LOC], f32))
        vnorm_sb = E(nc.sbuf_tensor("vnorm_sb", [128, N_LOC], f32))
        sq_sb = E(nc.sbuf_tensor("sq_sb", [128, 2, PH2G * 96], f32))
        vsq_sb = E(nc.sbuf_tensor("vsq_sb", [128, 2, PH2G * WIN], f32))
        eps_sb = E(nc.sbuf_tensor("eps_sb", [128, 1], f32))
        psum1 = [E(nc.psum_tensor(f"ps1_{i}", [128, 128], f32)) for i in range(NPSUM1)]
        psum2 = [E(nc.psum_tensor(f"ps2_{i}", [128, PH2G * 96], f32)) for i in range(2)]

        io = E(nc.semaphore("io"))
        gisem = E(nc.semaphore("gisem"))
        gsems = [E(nc.semaphore(f"gsem{q}")) for q in range(N_Q)]
        dve_sem = E(nc.semaphore("dve_sem"))
        pe_win = E(nc.semaphore("pe_win"))
        evac = E(nc.semaphore("evac"))
        pe2 = E(nc.semaphore("pe2"))
        sqs = E(nc.semaphore("sqs"))
        vsqs = E(nc.semaphore("vsqs"))
        vns = E(nc.semaphore("vns"))
        outs = E(nc.semaphore("outs"))

        N_IN_DMAS = 7
        IO_READY = 16 * N_IN_DMAS

        @block.gpsimd
        def _(g: bass.BassGpSimd):
            g.load_library(mlp)
            cpg = GCH // 128   # sbuf chunks per gather
            ipg = GCH // 16    # idx cols per gather
            per_piece = -(-N_GATHER // N_Q)  # gidx arrives in N_Q pieces
            seen_piece = 0
            for s in range(N_GATHER):
                piece = s // per_piece + 1
                if piece > seen_piece:
                    g.wait_ge(gisem, 16 * piece)
                    seen_piece = piece
                q = _gq(s)
                k = s // N_Q       # index within queue q's stream
                if k >= QDEPTH:
                    g.wait_ge(gsems[q], 16 * (k - QDEPTH + 1))
                g.dma_gather(
                    gath[:, s * cpg:(s + 1) * cpg, :],
                    table[:],
                    gidx_sb[:, s * ipg:(s + 1) * ipg],
                    GCH, GCH, F,
                    queue_num=q,
                ).then_inc(gsems[q], 16)

        @block.vector
        def _(v: bass.BassVectorEngine):
            v.memset(eps_sb[:], EPS)
            v.wait_ge(io, IO_READY)
            for w in range(N_WIN):
                if w >= NWONE:
                    v.wait_ge(pe_win, w - NWONE + 1)
                # eq[p, k, a] = (a32[p, w*K_CH+k] == iota[a])
                v.tensor_tensor(
                    out=eq_sb[:],
                    in0=a32_sb[:, K_CH * w:K_CH * (w + 1), None].to_broadcast(
                        [128, K_CH, WIN]),
                    in1=iota_sb[:],
                    op=mybir.AluOpType.is_equal,
                )
                # wone[p, k, ch, a] = eq[p, k, a] * coef[p, 5w+k, ch]
                v.tensor_tensor(
                    out=wone_sb[:, w % NWONE].rearrange("p k (c a) -> p k c a", c=4),
                    in0=eq_sb[:, :, None, :].to_broadcast([128, K_CH, 4, WIN]),
                    in1=coef_sb[:, K_CH * w:K_CH * (w + 1), :, None].to_broadcast(
                        [128, K_CH, 4, WIN]),
                    op=mybir.AluOpType.mult,
                ).then_inc(dve_sem, 1)
                if w % PH2G == PH2G - 1:
                    gi = w // PH2G
                    if gi >= 2:
                        v.wait_ge(vns, gi - 1)
                    v.wait_ge(sqs, gi + 1)
                    s3 = sq_sb[:, gi % 2].rearrange("p (w c a) -> p w c a", c=3, a=WIN)
                    v.tensor_tensor(
                        out=vsq_sb[:, gi % 2].rearrange("p (w a) -> p w a", a=WIN),
                        in0=s3[:, :, 0, :],
                        in1=s3[:, :, 1, :],
                        op=mybir.AluOpType.add,
                    )
                    v.tensor_tensor(
                        out=vsq_sb[:, gi % 2].rearrange("p (w a) -> p w a", a=WIN),
                        in0=vsq_sb[:, gi % 2].rearrange("p (w a) -> p w a", a=WIN),
                        in1=s3[:, :, 2, :],
                        op=mybir.AluOpType.add,
                    ).then_inc(vsqs, 1)

        @block.tensor
        def _(t: bass.BassTensorEngine):
            waited = [0] * N_Q
            for w in range(N_WIN):
                ps = psum1[w % NPSUM1]
                t.wait_ge(dve_sem, w + 1)
                last_g = (K_CH * w + K_CH - 1) // (GCH // 128)
                for q in range(N_Q):
                    cnt = _qcnt(last_g, q)
                    if cnt > waited[q]:
                        t.wait_ge(gsems[q], 16 * cnt)
                        waited[q] = cnt
                if w >= NPSUM1:
                    t.wait_ge(evac, 2 * (w - NPSUM1 + 1))
                for k in range(K_CH):
                    mm = t.matmul(
                        out=ps[:],
                        lhsT=gath[:, K_CH * w + k, :],
                        rhs=wone_sb[:, w % NWONE, k, :],
                        start=(k == 0),
                        stop=(k == K_CH - 1),
                    )
                    if k == K_CH - 1:
                        mm.then_inc(pe_win, 1)
                if w % PH2G == PH2G - 1:
                    gi = w // PH2G
                    t.wait_ge(evac, 2 * PH2G * (gi + 1))
                    if gi >= 2:
                        t.wait_ge(sqs, gi - 1)
                    t.matmul(
                        out=psum2[gi % 2][:],
                        lhsT=wt_sb[:],
                        rhs=vec_sb[:, gi * PH2G * 96:(gi + 1) * PH2G * 96],
                        start=True, stop=False,
                    )
                    t.matmul(
                        out=psum2[gi % 2][:],
                        lhsT=bvec_sb[:],
                        rhs=cnt3_sb[:, gi * PH2G * 96:(gi + 1) * PH2G * 96],
                        start=False, stop=True,
                    ).then_inc(pe2, 1)

        @block.scalar
        def _(a: bass.BassEngine):
            Copy = mybir.ActivationFunctionType.Copy
            for w in range(N_WIN):
                a.wait_ge(pe_win, w + 1)
                ps = psum1[w % NPSUM1]
                a.activation(out=vec_sb[:, w * 96:(w + 1) * 96],
                             in_=ps[:, WIN:128], func=Copy).then_inc(evac, 1)
                a.activation(out=rad_sb[:, w * WIN:(w + 1) * WIN],
                             in_=ps[:, 0:WIN], func=Copy).then_inc(evac, 1)
                if w % PH2G == PH2G - 1:
                    gi = w // PH2G
                    a.wait_ge(pe2, gi + 1)
                    if gi >= 2:
                        a.wait_ge(vsqs, gi - 1)
                    a.activation(out=sq_sb[:, gi % 2], in_=psum2[gi % 2][:],
                                 func=mybir.ActivationFunctionType.Square,
                                 ).then_inc(sqs, 1)
                    a.wait_ge(vsqs, gi + 1)
                    a.activation(out=vnorm_sb[:, gi * PH2G * WIN:(gi + 1) * PH2G * WIN],
                                 in_=vsq_sb[:, gi % 2],
                                 func=mybir.ActivationFunctionType.Sqrt,
                                 bias=eps_sb[:, 0:1]).then_inc(vns, 1)

        @block.sync
        def _(s: bass.BassEngine):
            ipg = GCH // 16
            per_piece = -(-N_GATHER // N_Q)
            for k in range(N_Q):
                lo = k * per_piece * ipg
                hi = min((k + 1) * per_piece, N_GATHER) * ipg
                s.dma_start(gidx_sb[:, lo:hi], gidx[:, lo:hi]).then_inc(gisem, 16)
            s.dma_start(a32_sb[:], a32[:]).then_inc(io, 16)
            s.dma_start(coef_sb[:], coef[:]).then_inc(io, 16)
            s.dma_start(iota_sb[:], iota[:]).then_inc(io, 16)
            s.dma_start(wt_sb[:], wt[:]).then_inc(io, 16)
            s.dma_start(bvec_sb[:], bvec[:]).then_inc(io, 16)
            s.dma_start(cnt3_sb[:], cnt3[:]).then_inc(io, 16)
            s.dma_start(wcnt_sb[:], wcnt[:]).then_inc(io, 16)
            for gi in range(N_PH2):
                c0, c1 = gi * PH2G * WIN, (gi + 1) * PH2G * WIN
                s.wait_ge(evac, 2 * PH2G * (gi + 1))
                s.dma_start(out[128:256, c0:c1], rad_sb[:, c0:c1]).then_inc(outs, 16)
                s.wait_ge(vns, gi + 1)
                s.dma_start(out[0:128, c0:c1], vnorm_sb[:, c0:c1]).then_inc(outs, 16)
            s.wait_ge(outs, 32 * N_PH2)

    nc.compile()
    return nc


def _prep_core(idx_i, idx_j, coef4, base):
    """Build per-core host arrays. idx_* already filtered+sorted by idx_i.

    Greedy variable-atom windows: consecutive local atoms are packed into a
    window until it would exceed SLOTS_PER_WIN pairs or WIN atoms."""
    a_loc = idx_i - base                       # [p] in [0, ATOMS_PER_CORE)
    counts = np.bincount(a_loc, minlength=ATOMS_PER_CORE)
    atom_win = np.zeros(ATOMS_PER_CORE, dtype=np.int64)
    atom_rank = np.zeros(ATOMS_PER_CORE, dtype=np.int64)
    w = acc = na = 0
    for atom in range(ATOMS_PER_CORE):
        c = int(counts[atom])
        if acc + c > SLOTS_PER_WIN or na == WIN:
            w += 1
            acc = na = 0
        atom_win[atom] = w
        atom_rank[atom] = na
        acc += c
        na += 1
    if w >= N_WIN:
        raise RuntimeError(f"needs {w + 1} windows > {N_WIN}")
    win = atom_win[a_loc]
    jidx = np.zeros(N_SLOT, dtype=np.int16)
    a32v = np.zeros(N_SLOT, dtype=np.float32)
    cf = np.zeros((N_SLOT, 4), dtype=np.float32)
    cnt_w = np.bincount(win, minlength=N_WIN)
    # pairs are sorted by idx_i hence grouped by window
    starts_in = np.concatenate([[0], np.cumsum(cnt_w)[:-1]])
    for wi in range(N_WIN):
        n = cnt_w[wi]
        if n == 0:
            continue
        s0, d0 = starts_in[wi], wi * SLOTS_PER_WIN
        jidx[d0:d0 + n] = idx_j[s0:s0 + n]
        a32v[d0:d0 + n] = atom_rank[a_loc[s0:s0 + n]].astype(np.float32)
        cf[d0:d0 + n] = coef4[s0:s0 + n]
    # gather idx wrap: per window-call, [16, GCH//16] blocks
    gidx_h = np.tile(
        jidx.reshape(N_GATHER, GCH // 16, 16).transpose(2, 0, 1).reshape(16, -1),
        (8, 1))
    wcnt_h = cnt_w.astype(np.int32).reshape(1, -1)
    # slot -> (partition, chunk): slot = chunk*128 + p
    a32_h = a32v.reshape(N_CHUNK, 128).T.astype(ml_dtypes.bfloat16)
    coef_h = np.ascontiguousarray(
        cf.reshape(N_CHUNK, 128, 4).transpose(1, 0, 2)).astype(ml_dtypes.bfloat16)
    # counts replicated over 3 vec channels: [w, c, a-rank]
    col_of = (atom_win * WIN + atom_rank).astype(np.int64)
    cnts_col = np.zeros(N_LOC, dtype=np.float32)
    cnts_col[col_of] = counts
    cnt3_h = np.broadcast_to(
        cnts_col.reshape(N_WIN, 1, WIN), (N_WIN, 3, WIN)).reshape(1, -1)
    return (gidx_h, a32_h, coef_h,
            np.ascontiguousarray(cnt3_h).astype(ml_dtypes.bfloat16), wcnt_h, col_of)


def _windows_needed(a_loc):
    counts = np.bincount(a_loc, minlength=ATOMS_PER_CORE)
    w = acc = na = 0
    for atom in range(ATOMS_PER_CORE):
        c = int(counts[atom])
        if acc + c > SLOTS_PER_WIN or na == WIN:
            w += 1
            acc = na = 0
        acc += c
        na += 1
    return w + 1


def _set_n_win(nw):
    g = globals()
    g["N_WIN"] = nw
    g["N_LOC"] = nw * WIN
    g["N_SLOT"] = nw * SLOTS_PER_WIN
    g["N_CHUNK"] = g["N_SLOT"] // 128
    g["N_GATHER"] = g["N_SLOT"] // GCH
    g["N_PH2"] = nw // PH2G


def kernel(atomic_embedding, pairlist, f_ij_cutoff, r_ij, W, b):
    atomic_embedding = np.asarray(atomic_embedding, dtype=np.float32)
    pairlist = np.asarray(pairlist)
    f_ij = np.asarray(f_ij_cutoff, dtype=np.float32).reshape(-1)
    r_ij = np.asarray(r_ij, dtype=np.float32)
    W = np.asarray(W, dtype=np.float32)
    b = np.asarray(b, dtype=np.float32)

    u = r_ij / np.linalg.norm(r_ij, axis=1, keepdims=True)
    coef4 = np.concatenate([f_ij[:, None], f_ij[:, None] * u], axis=1)  # [P,4]

    idx_i = np.asarray(pairlist[0], dtype=np.int64)
    idx_j = np.asarray(pairlist[1], dtype=np.int64)
    order = np.argsort(idx_i, kind="stable")
    idx_i_s, idx_j_s, coef_s = idx_i[order], idx_j[order], coef4[order]

    table = atomic_embedding.astype(ml_dtypes.bfloat16)
    iota_h = np.broadcast_to(
        np.arange(WIN, dtype=np.float32), (128, K_CH, WIN))
    iota_h = np.ascontiguousarray(iota_h).astype(ml_dtypes.bfloat16)
    wt_h = np.ascontiguousarray(W.T).astype(ml_dtypes.bfloat16)
    b_h = b.reshape(1, F).astype(ml_dtypes.bfloat16)

    bounds = np.searchsorted(idx_i_s, np.arange(0, N_ATOMS + 1, ATOMS_PER_CORE))
    need = max(_windows_needed(idx_i_s[bounds[c]:bounds[c + 1]] - c * ATOMS_PER_CORE)
               for c in range(N_CORES))
    # round up: multiple of PH2G (phase-2 groups) and of 2 (1024-idx gathers)
    nw = -(-max(need, 16) // 2) * 2
    if nw != N_WIN:
        _cache.pop("nc", None)
    _set_n_win(nw)
    in_maps = []
    colmaps = []
    for c in range(N_CORES):
        lo, hi = bounds[c], bounds[c + 1]
        gidx_h, a32_h, coef_h, cnt3_h, wcnt_h, col_of = _prep_core(
            idx_i_s[lo:hi], idx_j_s[lo:hi], coef_s[lo:hi], c * ATOMS_PER_CORE)
        in_maps.append({
            "table": table, "gidx": gidx_h, "a32": a32_h, "coef": coef_h,
            "iota": iota_h, "wt": wt_h, "bvec": b_h, "cnt3": cnt3_h,
            "wcnt": wcnt_h,
        })
        colmaps.append(col_of)

    if "nc" not in _cache:
        _cache["nc"] = _build_graph()
    res = run_bass_kernel_spmd(_cache["nc"], in_maps, core_ids=list(range(N_CORES)))

    out_full = np.empty((N_ATOMS, 2 * F), dtype=np.float32)
    for c in range(N_CORES):
        o = res.results[c]["out"]  # [256, N_LOC]
        n = ATOMS_PER_CORE
        out_full[c * n:(c + 1) * n, :] = o[:, colmaps[c]].T
    return out_full



# revision 14
# speedup vs baseline: 2.9982x; 1.3572x over previous
"""AIMNet2 interaction module on 8 TRN2 NeuronCores.

Strategy
--------
Algebraic restructure: the nn.Linear commutes with the segment-sum, so we
accumulate A[n, ch, f] = sum_{p: idx_i[p]=n} c_ch[p] * E[idx_j[p], f] with
c = f_ij * [1, ux, uy, uz] (4 channels), then apply W on the [N,3,F] result
plus a count_n * b correction, then the norm.  This cuts matmul work 16x and
avoids materializing [P,3,F].

Sharding: pairs sorted by idx_i on host; each core owns a contiguous range of
2500 target atoms and all pairs whose idx_i lands in it -> zero inter-core
communication.  Atoms are greedy-packed into windows (<=32 atoms AND <=512
pairs each; ~84 windows/core, adaptive, multiple of 4).

Per-core device pipeline (raw bass, manual semaphores):
  gpsimd : dma_gather calls (1024 rows x 256B bf16) of neighbor embeddings,
           round-robin over 4 SWDGE queues so desc-gen runs on all 4 Q7 cpu
           pairs concurrently (~4x the single-queue desc-gen rate); this
           stream sets the kernel runtime (~2.2us per 1024 rows).
  PE     : per 128-pair chunk: bank[f, (w%4)*128+(ch,a)] += a_j^T @ wone
           (4 windows share a 512-col psum bank); per 4-window group the
           W-transform psum2[g, (w,c,a)] = wt.T @ vec + b x counts.
  ACT    : one full-bank evac per group (f32->bf16, rad+vec together),
           Square, Sqrt.
  DVE    : per-group sum-of-squares adds only (wone comes precomputed from
           the host, streamed via DMA into a 4-group SBUF ring).
  sync   : input DMAs (gidx in 4 pieces, wone in per-group pieces) + per-group
           streamed output DMAs.
"""
import sys
import numpy as np
import ml_dtypes

sys.path.insert(0, "/opt/trn_rl_repo")

import concourse.bass as bass
import concourse.bacc as bacc
import concourse.mybir as mybir
from concourse.bass_utils import run_bass_kernel_spmd
from concourse.library_config import mlp

# ---------------- problem constants (hardcoded per spec) ----------------
N_ATOMS = 20000
F = 128
N_CORES = 8
ATOMS_PER_CORE = 2500          # 8 * 2500 = 20000
WIN = 32                       # max atoms per window
N_WIN = 84                     # windows per core (greedy-packed; adaptive)
N_LOC = N_WIN * WIN            # padded local atom slots
K_CH = 4                       # chunks (x128 pairs) per window -> 512 slots
SLOTS_PER_WIN = K_CH * 128
N_SLOT = N_WIN * SLOTS_PER_WIN
N_CHUNK = N_SLOT // 128
GCH = 1024                     # rows per dma_gather call
N_GATHER = N_SLOT // GCH
GRP = 4                        # windows per group (psum bank / phase-2 unit)
N_GRP = N_WIN // GRP
WRING = 4                      # wone ring depth, in groups
EPS = 1e-12

N_Q = 4                        # SWDGE queues (desc-gen cpu pairs), round-robin
QDEPTH = 4                     # outstanding gathers per queue (16 total)

bf16 = mybir.dt.bfloat16
f32 = mybir.dt.float32
i16 = mybir.dt.int16

_cache = {}


def _gq(s):
    return s % N_Q


def _qcnt(s, q):
    """#gathers with index <= s on queue q."""
    if s < q:
        return 0
    return (s - q) // N_Q + 1


def _build_graph():
    nc = bacc.Bacc("TRN2", debug=False, num_swdge_queues=N_Q)
    dp = nc.declare_dram_parameter
    table = dp("table", [N_ATOMS, F], bf16, isOutput=False)
    gidx = dp("gidx", [128, N_SLOT // 16], i16, isOutput=False)
    wone = dp("wone", [128, N_WIN, K_CH, 128], bf16, isOutput=False)
    wt = dp("wt", [F, F], bf16, isOutput=False)          # W transposed
    bvec = dp("bvec", [1, F], bf16, isOutput=False)
    cnt3 = dp("cnt3", [1, N_WIN * 96], bf16, isOutput=False)
    out_v = dp("out_v", [128, N_LOC], f32, isOutput=True)   # vector norms
    out_r = dp("out_r", [128, N_LOC], bf16, isOutput=True)  # radial

    import contextlib
    with contextlib.ExitStack() as ctx:
        E = ctx.enter_context
        block = E(nc.Block())
        gath = E(nc.sbuf_tensor("gath", [128, N_CHUNK, F], bf16))
        gidx_sb = E(nc.sbuf_tensor("gidx_sb", [128, N_SLOT // 16], i16))
        wone_sb = E(nc.sbuf_tensor("wone_sb", [128, WRING, GRP, K_CH, 128], bf16))
        wt_sb = E(nc.sbuf_tensor("wt_sb", [F, F], bf16))
        bvec_sb = E(nc.sbuf_tensor("bvec_sb", [1, F], bf16))
        cnt3_sb = E(nc.sbuf_tensor("cnt3_sb", [1, N_WIN * 96], bf16))
        # per-group evac: [rad(32)|vec(96)] x 4 windows, bf16
        mixed_sb = E(nc.sbuf_tensor("mixed_sb", [128, N_WIN * 128], bf16))
        vnorm_sb = E(nc.sbuf_tensor("vnorm_sb", [128, N_LOC], f32))
        sq_sb = E(nc.sbuf_tensor("sq_sb", [128, 2, GRP * 96], f32))
        vsq_sb = E(nc.sbuf_tensor("vsq_sb", [128, 2, GRP * WIN], f32))
        eps_sb = E(nc.sbuf_tensor("eps_sb", [128, 1], f32))
        banks = [E(nc.psum_tensor(f"bank{i}", [128, GRP * 128], f32))
                 for i in range(4)]
        psum2 = [E(nc.psum_tensor(f"ps2_{i}", [128, GRP * 96], f32))
                 for i in range(2)]

        io = E(nc.semaphore("io"))
        # per-piece sems: concurrent DMAs complete out of order, so a single
        # counting semaphore cannot express "pieces 0..k landed"
        gisems = [E(nc.semaphore(f"gisem{k}")) for k in range(N_Q)]
        gsems = [E(nc.semaphore(f"gsem{q}")) for q in range(N_Q)]
        wsems = [E(nc.semaphore(f"wsem{k}")) for k in range(WRING)]
        pe_win = E(nc.semaphore("pe_win"))
        evac = E(nc.semaphore("evac"))
        pe2 = E(nc.semaphore("pe2"))
        sqs = E(nc.semaphore("sqs"))
        vsqs = E(nc.semaphore("vsqs"))
        vns = E(nc.semaphore("vns"))
        outs = E(nc.semaphore("outs"))

        @block.gpsimd
        def _(g: bass.BassGpSimd):
            g.load_library(mlp)
            cpg = GCH // 128   # sbuf chunks per gather
            ipg = GCH // 16    # idx cols per gather
            per_piece = -(-N_GATHER // N_Q)  # gidx arrives in N_Q pieces
            seen_piece = -1
            for s in range(N_GATHER):
                piece = s // per_piece
                if piece > seen_piece:
                    g.wait_ge(gisems[piece], 16)
                    seen_piece = piece
                q = _gq(s)
                k = s // N_Q
                if k >= QDEPTH:
                    g.wait_ge(gsems[q], 16 * (k - QDEPTH + 1))
                g.dma_gather(
                    gath[:, s * cpg:(s + 1) * cpg, :],
                    table[:],
                    gidx_sb[:, s * ipg:(s + 1) * ipg],
                    GCH, GCH, F,
                    queue_num=q,
                ).then_inc(gsems[q], 16)

        @block.tensor
        def _(t: bass.BassTensorEngine):
            t.wait_ge(io, 16 * 3)   # wt, bvec, cnt3
            waited = [0] * N_Q
            for w in range(N_WIN):
                gi = w // GRP
                bank = banks[gi % 4]
                if w % GRP == 0:
                    t.wait_ge(wsems[gi % WRING], 16 * (gi // WRING + 1))
                    if gi >= 4:
                        t.wait_ge(evac, gi - 3)   # psum bank reuse
                last_g = (K_CH * w + K_CH - 1) // (GCH // 128)
                for q in range(N_Q):
                    cnt = _qcnt(last_g, q)
                    if cnt > waited[q]:
                        t.wait_ge(gsems[q], 16 * cnt)
                        waited[q] = cnt
                c0 = (w % GRP) * 128
                for k in range(K_CH):
                    mm = t.matmul(
                        out=bank[:, c0:c0 + 128],
                        lhsT=gath[:, K_CH * w + k, :],
                        rhs=wone_sb[:, gi % WRING, w % GRP, k, :],
                        start=(k == 0),
                        stop=(k == K_CH - 1),
                    )
                    if k == K_CH - 1:
                        mm.then_inc(pe_win, 1)
                if w % GRP == GRP - 1:
                    # phase 2 for this group
                    t.wait_ge(evac, gi + 1)
                    if gi >= 2:
                        t.wait_ge(sqs, gi - 1)    # psum2 slot reuse
                    mx = mixed_sb[:, gi * GRP * 128:(gi + 1) * GRP * 128]
                    vec = mx.rearrange("p (w c) -> p w c", w=GRP)[:, :, 32:128]
                    t.matmul(
                        out=psum2[gi % 2][:],
                        lhsT=wt_sb[:],
                        rhs=vec,
                        start=True, stop=False,
                    )
                    t.matmul(
                        out=psum2[gi % 2][:],
                        lhsT=bvec_sb[:],
                        rhs=cnt3_sb[:, gi * GRP * 96:(gi + 1) * GRP * 96],
                        start=False, stop=True,
                    ).then_inc(pe2, 1)

        @block.scalar
        def _(a: bass.BassEngine):
            Copy = mybir.ActivationFunctionType.Copy
            for gi in range(N_GRP):
                a.wait_ge(pe_win, GRP * (gi + 1))
                a.activation(
                    out=mixed_sb[:, gi * GRP * 128:(gi + 1) * GRP * 128],
                    in_=banks[gi % 4][:], func=Copy).then_inc(evac, 1)
                a.wait_ge(pe2, gi + 1)
                if gi >= 2:
                    a.wait_ge(vsqs, gi - 1)      # sq slot reuse
                a.activation(out=sq_sb[:, gi % 2], in_=psum2[gi % 2][:],
                             func=mybir.ActivationFunctionType.Square,
                             ).then_inc(sqs, 1)
                a.wait_ge(vsqs, gi + 1)
                a.activation(out=vnorm_sb[:, gi * GRP * WIN:(gi + 1) * GRP * WIN],
                             in_=vsq_sb[:, gi % 2],
                             func=mybir.ActivationFunctionType.Sqrt,
                             bias=eps_sb[:, 0:1]).then_inc(vns, 1)

        @block.vector
        def _(v: bass.BassVectorEngine):
            v.memset(eps_sb[:], EPS)
            for gi in range(N_GRP):
                v.wait_ge(sqs, gi + 1)
                if gi >= 2:
                    v.wait_ge(vns, gi - 1)       # vsq slot reuse
                s3 = sq_sb[:, gi % 2].rearrange("p (w c a) -> p w c a",
                                                c=3, a=WIN)
                v.tensor_tensor(
                    out=vsq_sb[:, gi % 2].rearrange("p (w a) -> p w a", a=WIN),
                    in0=s3[:, :, 0, :],
                    in1=s3[:, :, 1, :],
                    op=mybir.AluOpType.add,
                )
                v.tensor_tensor(
                    out=vsq_sb[:, gi % 2].rearrange("p (w a) -> p w a", a=WIN),
                    in0=vsq_sb[:, gi % 2].rearrange("p (w a) -> p w a", a=WIN),
                    in1=s3[:, :, 2, :],
                    op=mybir.AluOpType.add,
                ).then_inc(vsqs, 1)

        @block.sync
        def _(s: bass.BassEngine):
            ipg = GCH // 16
            per_piece = -(-N_GATHER // N_Q)
            for k in range(N_Q):
                lo = k * per_piece * ipg
                hi = min((k + 1) * per_piece, N_GATHER) * ipg
                s.dma_start(gidx_sb[:, lo:hi], gidx[:, lo:hi]
                            ).then_inc(gisems[k], 16)
            s.dma_start(wt_sb[:], wt[:]).then_inc(io, 16)
            s.dma_start(bvec_sb[:], bvec[:]).then_inc(io, 16)
            s.dma_start(cnt3_sb[:], cnt3[:]).then_inc(io, 16)
            def wone_piece(slot, g0):
                s.dma_start(
                    wone_sb[:, slot].rearrange("p a b c -> p (a b c)"),
                    wone[:, g0 * GRP:(g0 + 1) * GRP].rearrange(
                        "p a b c -> p (a b c)"),
                ).then_inc(wsems[slot], 16)

            for gi in range(min(WRING, N_GRP)):
                wone_piece(gi, gi)
            for gi in range(N_GRP):
                nxt = gi + WRING
                if nxt < N_GRP:
                    # wone ring slot reuse: group nxt-WRING fully consumed
                    s.wait_ge(pe_win, GRP * (nxt - WRING + 1))
                    wone_piece(nxt % WRING, nxt)
                c0, c1 = gi * GRP * WIN, (gi + 1) * GRP * WIN
                s.wait_ge(evac, gi + 1)
                mx = mixed_sb[:, gi * GRP * 128:(gi + 1) * GRP * 128]
                rad = mx.rearrange("p (w c) -> p w c", w=GRP)[:, :, 0:32]
                s.dma_start(out_r[:, c0:c1], rad).then_inc(outs, 16)
                s.wait_ge(vns, gi + 1)
                s.dma_start(out_v[:, c0:c1], vnorm_sb[:, c0:c1]
                            ).then_inc(outs, 16)
            s.wait_ge(outs, 32 * N_GRP)

    nc.compile()
    return nc


def _prep_core(idx_i, idx_j, coef4, base):
    """Build per-core host arrays. idx_* already filtered+sorted by idx_i.

    Greedy variable-atom windows: consecutive local atoms are packed into a
    window until it would exceed SLOTS_PER_WIN pairs or WIN atoms."""
    a_loc = idx_i - base                       # [p] in [0, ATOMS_PER_CORE)
    counts = np.bincount(a_loc, minlength=ATOMS_PER_CORE)
    atom_win = np.zeros(ATOMS_PER_CORE, dtype=np.int64)
    atom_rank = np.zeros(ATOMS_PER_CORE, dtype=np.int64)
    w = acc = na = 0
    for atom in range(ATOMS_PER_CORE):
        c = int(counts[atom])
        if acc + c > SLOTS_PER_WIN or na == WIN:
            w += 1
            acc = na = 0
        atom_win[atom] = w
        atom_rank[atom] = na
        acc += c
        na += 1
    if w >= N_WIN:
        raise RuntimeError(f"needs {w + 1} windows > {N_WIN}")
    win = atom_win[a_loc]
    jidx = np.zeros(N_SLOT, dtype=np.int16)
    slot_rank = np.zeros(N_SLOT, dtype=np.int64)
    slot_coef = np.zeros((N_SLOT, 4), dtype=np.float32)
    cnt_w = np.bincount(win, minlength=N_WIN)
    # pairs are sorted by idx_i hence grouped by window
    starts_in = np.concatenate([[0], np.cumsum(cnt_w)[:-1]])
    for wi in range(N_WIN):
        n = cnt_w[wi]
        if n == 0:
            continue
        s0, d0 = starts_in[wi], wi * SLOTS_PER_WIN
        jidx[d0:d0 + n] = idx_j[s0:s0 + n]
        slot_rank[d0:d0 + n] = atom_rank[a_loc[s0:s0 + n]]
        slot_coef[d0:d0 + n] = coef4[s0:s0 + n]
    # gather idx wrap: per gather-call, [16, GCH//16] blocks
    gidx_h = np.tile(
        jidx.reshape(N_GATHER, GCH // 16, 16).transpose(2, 0, 1).reshape(16, -1),
        (8, 1))
    # weighted one-hot rhs, precomputed: [p, win, k, (c, a)]
    wone_flat = np.zeros((N_SLOT, 4, WIN), dtype=np.float32)
    wone_flat[np.arange(N_SLOT), :, slot_rank] = slot_coef
    wone_h = np.ascontiguousarray(
        wone_flat.reshape(N_WIN, K_CH, 128, 4 * WIN).transpose(2, 0, 1, 3)
    ).astype(ml_dtypes.bfloat16)
    # counts replicated over 3 vec channels: [w, c, a-rank]
    col_of = (atom_win * WIN + atom_rank).astype(np.int64)
    cnts_col = np.zeros(N_LOC, dtype=np.float32)
    cnts_col[col_of] = counts
    cnt3_h = np.broadcast_to(
        cnts_col.reshape(N_WIN, 1, WIN), (N_WIN, 3, WIN)).reshape(1, -1)
    return (gidx_h, wone_h,
            np.ascontiguousarray(cnt3_h).astype(ml_dtypes.bfloat16), col_of)


def _windows_needed(a_loc):
    counts = np.bincount(a_loc, minlength=ATOMS_PER_CORE)
    w = acc = na = 0
    for atom in range(ATOMS_PER_CORE):
        c = int(counts[atom])
        if acc + c > SLOTS_PER_WIN or na == WIN:
            w += 1
            acc = na = 0
        acc += c
        na += 1
    return w + 1


def _set_n_win(nw):
    g = globals()
    g["N_WIN"] = nw
    g["N_LOC"] = nw * WIN
    g["N_SLOT"] = nw * SLOTS_PER_WIN
    g["N_CHUNK"] = g["N_SLOT"] // 128
    g["N_GATHER"] = g["N_SLOT"] // GCH
    g["N_GRP"] = nw // GRP


def kernel(atomic_embedding, pairlist, f_ij_cutoff, r_ij, W, b):
    atomic_embedding = np.asarray(atomic_embedding, dtype=np.float32)
    pairlist = np.asarray(pairlist)
    f_ij = np.asarray(f_ij_cutoff, dtype=np.float32).reshape(-1)
    r_ij = np.asarray(r_ij, dtype=np.float32)
    W = np.asarray(W, dtype=np.float32)
    b = np.asarray(b, dtype=np.float32)

    u = r_ij / np.linalg.norm(r_ij, axis=1, keepdims=True)
    coef4 = np.concatenate([f_ij[:, None], f_ij[:, None] * u], axis=1)  # [P,4]

    idx_i = np.asarray(pairlist[0], dtype=np.int64)
    idx_j = np.asarray(pairlist[1], dtype=np.int64)
    order = np.argsort(idx_i, kind="stable")
    idx_i_s, idx_j_s, coef_s = idx_i[order], idx_j[order], coef4[order]

    table = atomic_embedding.astype(ml_dtypes.bfloat16)
    wt_h = np.ascontiguousarray(W.T).astype(ml_dtypes.bfloat16)
    b_h = b.reshape(1, F).astype(ml_dtypes.bfloat16)

    bounds = np.searchsorted(idx_i_s, np.arange(0, N_ATOMS + 1, ATOMS_PER_CORE))
    need = max(_windows_needed(idx_i_s[bounds[c]:bounds[c + 1]] - c * ATOMS_PER_CORE)
               for c in range(N_CORES))
    # round up: multiple of GRP (phase-2 groups) and of 2 (1024-idx gathers)
    nw = -(-max(need, 16) // GRP) * GRP
    if nw != N_WIN:
        _cache.pop("nc", None)
    _set_n_win(nw)
    in_maps = []
    colmaps = []
    for c in range(N_CORES):
        lo, hi = bounds[c], bounds[c + 1]
        gidx_h, wone_h, cnt3_h, col_of = _prep_core(
            idx_i_s[lo:hi], idx_j_s[lo:hi], coef_s[lo:hi], c * ATOMS_PER_CORE)
        in_maps.append({
            "table": table, "gidx": gidx_h,
            "wone": wone_h.reshape(128, N_WIN, K_CH, 128),
            "wt": wt_h, "bvec": b_h, "cnt3": cnt3_h,
        })
        colmaps.append(col_of)

    if "nc" not in _cache:
        _cache["nc"] = _build_graph()
    res = run_bass_kernel_spmd(_cache["nc"], in_maps, core_ids=list(range(N_CORES)))

    out_full = np.empty((N_ATOMS, 2 * F), dtype=np.float32)
    for c in range(N_CORES):
        ov = res.results[c]["out_v"]                      # [128, N_LOC] f32
        orad = np.asarray(res.results[c]["out_r"]).astype(np.float32)
        n = ATOMS_PER_CORE
        out_full[c * n:(c + 1) * n, 0:F] = ov[:, colmaps[c]].T
        out_full[c * n:(c + 1) * n, F:] = orad[:, colmaps[c]].T
    return out_full


# revision 17
# speedup vs baseline: 3.0852x; 1.0290x over previous
"""AIMNet2 interaction module on 8 TRN2 NeuronCores.

Strategy
--------
Algebraic restructure: the nn.Linear commutes with the segment-sum, so we
accumulate A[n, ch, f] = sum_{p: idx_i[p]=n} c_ch[p] * E[idx_j[p], f] with
c = f_ij * [1, ux, uy, uz] (4 channels), then apply W on the [N,3,F] result
plus a count_n * b correction, then the norm.  This cuts matmul work 16x and
avoids materializing [P,3,F].

Sharding: pairs sorted by idx_i on host; each core owns a contiguous range of
2500 target atoms and all pairs whose idx_i lands in it -> zero inter-core
communication.  Atoms are greedy-packed into windows (<=32 atoms AND <=512
pairs each; ~84 windows/core, adaptive, multiple of 4).

Per-core device pipeline (raw bass, manual semaphores):
  gpsimd : dma_gather calls (1024 rows x 256B bf16) of neighbor embeddings,
           round-robin over 4 SWDGE queues so desc-gen runs on all 4 Q7 cpu
           pairs concurrently (~4x the single-queue desc-gen rate); this
           stream sets the kernel runtime (~2.2us per 1024 rows).
  PE     : per 128-pair chunk: bank[f, (w%4)*128+(ch,a)] += a_j^T @ wone
           (4 windows share a 512-col psum bank); per 4-window group the
           W-transform psum2[g, (w,c,a)] = wt.T @ vec + b x counts.
  ACT    : one full-bank evac per group (f32->bf16, rad+vec together),
           Square, Sqrt.
  DVE    : per-group sum-of-squares adds only (wone comes precomputed from
           the host, streamed via DMA into a 4-group SBUF ring).
  sync   : input DMAs (gidx in 4 pieces, wone in per-group pieces) + per-group
           streamed output DMAs.
"""
import sys
import numpy as np
import ml_dtypes

sys.path.insert(0, "/opt/trn_rl_repo")

import concourse.bass as bass
import concourse.bacc as bacc
import concourse.mybir as mybir
from concourse.bass_utils import run_bass_kernel_spmd
from concourse.library_config import mlp

# ---------------- problem constants (hardcoded per spec) ----------------
N_ATOMS = 20000
F = 128
N_CORES = 8
ATOMS_PER_CORE = 2500          # 8 * 2500 = 20000
WIN = 32                       # max atoms per window
N_WIN = 84                     # windows per core (greedy-packed; adaptive)
N_LOC = N_WIN * WIN            # padded local atom slots
K_CH = 4                       # chunks (x128 pairs) per window -> 512 slots
SLOTS_PER_WIN = K_CH * 128
N_SLOT = N_WIN * SLOTS_PER_WIN
N_CHUNK = N_SLOT // 128
GCH = 1024                     # rows per dma_gather call
N_GATHER = N_SLOT // GCH
GRP = 4                        # windows per group (psum bank / phase-2 unit)
N_GRP = N_WIN // GRP
WRING = 6                      # wone ring depth, in groups
EPS = 1e-12

N_Q = 4                        # SWDGE queues (desc-gen cpu pairs), round-robin
QDEPTH = 4                     # outstanding gathers per queue (16 total)

bf16 = mybir.dt.bfloat16
f32 = mybir.dt.float32
i16 = mybir.dt.int16

_cache = {}


def _gq(s):
    return s % N_Q


def _qcnt(s, q):
    """#gathers with index <= s on queue q."""
    if s < q:
        return 0
    return (s - q) // N_Q + 1


def _build_graph():
    nc = bacc.Bacc("TRN2", debug=False, num_swdge_queues=N_Q)
    dp = nc.declare_dram_parameter
    table = dp("table", [N_ATOMS, F], bf16, isOutput=False)
    gidx = dp("gidx", [128, N_SLOT // 16], i16, isOutput=False)
    wone = dp("wone", [128, N_WIN, K_CH, 128], bf16, isOutput=False)
    wt = dp("wt", [F, F], bf16, isOutput=False)          # W transposed
    bvec = dp("bvec", [1, F], bf16, isOutput=False)
    cnt3 = dp("cnt3", [1, N_WIN * 96], bf16, isOutput=False)
    out_v = dp("out_v", [128, N_LOC], f32, isOutput=True)   # vector norms
    out_r = dp("out_r", [128, N_LOC], bf16, isOutput=True)  # radial

    import contextlib
    with contextlib.ExitStack() as ctx:
        E = ctx.enter_context
        block = E(nc.Block())
        gath = E(nc.sbuf_tensor("gath", [128, N_CHUNK, F], bf16))
        gidx_sb = E(nc.sbuf_tensor("gidx_sb", [128, N_SLOT // 16], i16))
        wone_sb = E(nc.sbuf_tensor("wone_sb", [128, WRING, GRP, K_CH, 128], bf16))
        wt_sb = E(nc.sbuf_tensor("wt_sb", [F, F], bf16))
        bvec_sb = E(nc.sbuf_tensor("bvec_sb", [1, F], bf16))
        cnt3_sb = E(nc.sbuf_tensor("cnt3_sb", [1, N_WIN * 96], bf16))
        # per-group evac: [rad(32)|vec(96)] x 4 windows, bf16
        mixed_sb = E(nc.sbuf_tensor("mixed_sb", [128, N_WIN * 128], bf16))
        vnorm_sb = E(nc.sbuf_tensor("vnorm_sb", [128, N_LOC], f32))
        sq_sb = E(nc.sbuf_tensor("sq_sb", [128, 2, GRP * 96], f32))
        vsq_sb = E(nc.sbuf_tensor("vsq_sb", [128, 2, GRP * WIN], f32))
        eps_sb = E(nc.sbuf_tensor("eps_sb", [128, 1], f32))
        banks = [E(nc.psum_tensor(f"bank{i}", [128, GRP * 128], f32))
                 for i in range(4)]
        psum2 = [E(nc.psum_tensor(f"ps2_{i}", [128, GRP * 96], f32))
                 for i in range(2)]

        io = E(nc.semaphore("io"))
        # per-piece sems: concurrent DMAs complete out of order, so a single
        # counting semaphore cannot express "pieces 0..k landed"
        gisems = [E(nc.semaphore(f"gisem{k}")) for k in range(N_Q)]
        gsems = [E(nc.semaphore(f"gsem{q}")) for q in range(N_Q)]
        wsems = [E(nc.semaphore(f"wsem{k}")) for k in range(WRING)]
        pe_win = E(nc.semaphore("pe_win"))
        evac = E(nc.semaphore("evac"))
        pe2 = E(nc.semaphore("pe2"))
        sqs = E(nc.semaphore("sqs"))
        vsqs = E(nc.semaphore("vsqs"))
        vns = E(nc.semaphore("vns"))
        outs = E(nc.semaphore("outs"))

        @block.gpsimd
        def _(g: bass.BassGpSimd):
            g.load_library(mlp)
            nreg = g.to_reg(GCH)   # hoisted: one MOVE instead of one per call
            cpg = GCH // 128   # sbuf chunks per gather
            ipg = GCH // 16    # idx cols per gather
            per_piece = -(-N_GATHER // N_Q)  # gidx arrives in N_Q pieces
            seen_piece = -1
            for s in range(N_GATHER):
                piece = s // per_piece
                if piece > seen_piece:
                    g.wait_ge(gisems[piece], 16)
                    seen_piece = piece
                q = _gq(s)
                k = s // N_Q
                if k >= QDEPTH:
                    g.wait_ge(gsems[q], 16 * (k - QDEPTH + 1))
                g.dma_gather(
                    gath[:, s * cpg:(s + 1) * cpg, :],
                    table[:],
                    gidx_sb[:, s * ipg:(s + 1) * ipg],
                    GCH, nreg, F,
                    queue_num=q,
                ).then_inc(gsems[q], 16)

        @block.tensor
        def _(t: bass.BassTensorEngine):
            t.wait_ge(io, 16 * 3)   # wt, bvec, cnt3

            def phase2(g):
                t.wait_ge(evac, g + 1)
                if g >= 2:
                    t.wait_ge(sqs, g - 1)     # psum2 slot reuse
                mx = mixed_sb[:, g * GRP * 128:(g + 1) * GRP * 128]
                vec = mx.rearrange("p (w c) -> p w c", w=GRP)[:, :, 32:128]
                t.matmul(
                    out=psum2[g % 2][:],
                    lhsT=wt_sb[:],
                    rhs=vec,
                    start=True, stop=False,
                )
                t.matmul(
                    out=psum2[g % 2][:],
                    lhsT=bvec_sb[:],
                    rhs=cnt3_sb[:, g * GRP * 96:(g + 1) * GRP * 96],
                    start=False, stop=True,
                ).then_inc(pe2, 1)

            waited = [0] * N_Q
            for w in range(N_WIN):
                gi = w // GRP
                bank = banks[gi % 4]
                if w % GRP == 0:
                    t.wait_ge(wsems[gi % WRING], 16 * (gi // WRING + 1))
                    if gi >= 4:
                        t.wait_ge(evac, gi - 3)   # psum bank reuse
                last_g = (K_CH * w + K_CH - 1) // (GCH // 128)
                for q in range(N_Q):
                    cnt = _qcnt(last_g, q)
                    if cnt > waited[q]:
                        t.wait_ge(gsems[q], 16 * cnt)
                        waited[q] = cnt
                c0 = (w % GRP) * 128
                for k in range(K_CH):
                    mm = t.matmul(
                        out=bank[:, c0:c0 + 128],
                        lhsT=gath[:, K_CH * w + k, :],
                        rhs=wone_sb[:, gi % WRING, w % GRP, k, :],
                        start=(k == 0),
                        stop=(k == K_CH - 1),
                    )
                    if k == K_CH - 1:
                        mm.then_inc(pe_win, 1)
                # phase 2 lags one group so PE never idles on the evac chain
                if w % GRP == GRP - 1 and gi >= 1:
                    phase2(gi - 1)
            phase2(N_GRP - 1)

        @block.scalar
        def _(a: bass.BassEngine):
            Copy = mybir.ActivationFunctionType.Copy
            for gi in range(N_GRP):
                a.wait_ge(pe_win, GRP * (gi + 1))
                a.activation(
                    out=mixed_sb[:, gi * GRP * 128:(gi + 1) * GRP * 128],
                    in_=banks[gi % 4][:], func=Copy).then_inc(evac, 1)
                a.wait_ge(pe2, gi + 1)
                if gi >= 2:
                    a.wait_ge(vsqs, gi - 1)      # sq slot reuse
                a.activation(out=sq_sb[:, gi % 2], in_=psum2[gi % 2][:],
                             func=mybir.ActivationFunctionType.Square,
                             ).then_inc(sqs, 1)
                a.wait_ge(vsqs, gi + 1)
                a.activation(out=vnorm_sb[:, gi * GRP * WIN:(gi + 1) * GRP * WIN],
                             in_=vsq_sb[:, gi % 2],
                             func=mybir.ActivationFunctionType.Sqrt,
                             bias=eps_sb[:, 0:1]).then_inc(vns, 1)

        @block.vector
        def _(v: bass.BassVectorEngine):
            v.memset(eps_sb[:], EPS)
            for gi in range(N_GRP):
                v.wait_ge(sqs, gi + 1)
                if gi >= 2:
                    v.wait_ge(vns, gi - 1)       # vsq slot reuse
                s3 = sq_sb[:, gi % 2].rearrange("p (w c a) -> p w c a",
                                                c=3, a=WIN)
                v.tensor_tensor(
                    out=vsq_sb[:, gi % 2].rearrange("p (w a) -> p w a", a=WIN),
                    in0=s3[:, :, 0, :],
                    in1=s3[:, :, 1, :],
                    op=mybir.AluOpType.add,
                )
                v.tensor_tensor(
                    out=vsq_sb[:, gi % 2].rearrange("p (w a) -> p w a", a=WIN),
                    in0=vsq_sb[:, gi % 2].rearrange("p (w a) -> p w a", a=WIN),
                    in1=s3[:, :, 2, :],
                    op=mybir.AluOpType.add,
                ).then_inc(vsqs, 1)

        @block.sync
        def _(s: bass.BassEngine):
            ipg = GCH // 16
            per_piece = -(-N_GATHER // N_Q)
            for k in range(N_Q):
                lo = k * per_piece * ipg
                hi = min((k + 1) * per_piece, N_GATHER) * ipg
                s.dma_start(gidx_sb[:, lo:hi], gidx[:, lo:hi]
                            ).then_inc(gisems[k], 16)
            s.dma_start(wt_sb[:], wt[:]).then_inc(io, 16)
            s.dma_start(bvec_sb[:], bvec[:]).then_inc(io, 16)
            s.dma_start(cnt3_sb[:], cnt3[:]).then_inc(io, 16)
            def wone_piece(slot, g0):
                s.dma_start(
                    wone_sb[:, slot].rearrange("p a b c -> p (a b c)"),
                    wone[:, g0 * GRP:(g0 + 1) * GRP].rearrange(
                        "p a b c -> p (a b c)"),
                ).then_inc(wsems[slot], 16)

            for gi in range(min(WRING, N_GRP)):
                wone_piece(gi, gi)
            for gi in range(N_GRP):
                nxt = gi + WRING
                if nxt < N_GRP:
                    # wone ring slot reuse: group nxt-WRING fully consumed
                    s.wait_ge(pe_win, GRP * (nxt - WRING + 1))
                    wone_piece(nxt % WRING, nxt)
                c0, c1 = gi * GRP * WIN, (gi + 1) * GRP * WIN
                s.wait_ge(evac, gi + 1)
                mx = mixed_sb[:, gi * GRP * 128:(gi + 1) * GRP * 128]
                rad = mx.rearrange("p (w c) -> p w c", w=GRP)[:, :, 0:32]
                s.dma_start(out_r[:, c0:c1], rad).then_inc(outs, 16)
                s.wait_ge(vns, gi + 1)
                s.dma_start(out_v[:, c0:c1], vnorm_sb[:, c0:c1]
                            ).then_inc(outs, 16)
            s.wait_ge(outs, 32 * N_GRP)

    nc.compile()
    return nc


def _prep_core(idx_i, idx_j, coef4, base):
    """Build per-core host arrays. idx_* already filtered+sorted by idx_i.

    Greedy variable-atom windows: consecutive local atoms are packed into a
    window until it would exceed SLOTS_PER_WIN pairs or WIN atoms."""
    a_loc = idx_i - base                       # [p] in [0, ATOMS_PER_CORE)
    counts = np.bincount(a_loc, minlength=ATOMS_PER_CORE)
    atom_win = np.zeros(ATOMS_PER_CORE, dtype=np.int64)
    atom_rank = np.zeros(ATOMS_PER_CORE, dtype=np.int64)
    w = acc = na = 0
    for atom in range(ATOMS_PER_CORE):
        c = int(counts[atom])
        if acc + c > SLOTS_PER_WIN or na == WIN:
            w += 1
            acc = na = 0
        atom_win[atom] = w
        atom_rank[atom] = na
        acc += c
        na += 1
    if w >= N_WIN:
        raise RuntimeError(f"needs {w + 1} windows > {N_WIN}")
    win = atom_win[a_loc]
    jidx = np.zeros(N_SLOT, dtype=np.int16)
    slot_rank = np.zeros(N_SLOT, dtype=np.int64)
    slot_coef = np.zeros((N_SLOT, 4), dtype=np.float32)
    cnt_w = np.bincount(win, minlength=N_WIN)
    # pairs are sorted by idx_i hence grouped by window
    starts_in = np.concatenate([[0], np.cumsum(cnt_w)[:-1]])
    for wi in range(N_WIN):
        n = cnt_w[wi]
        if n == 0:
            continue
        s0, d0 = starts_in[wi], wi * SLOTS_PER_WIN
        jidx[d0:d0 + n] = idx_j[s0:s0 + n]
        slot_rank[d0:d0 + n] = atom_rank[a_loc[s0:s0 + n]]
        slot_coef[d0:d0 + n] = coef4[s0:s0 + n]
    # gather idx wrap: per gather-call, [16, GCH//16] blocks
    gidx_h = np.tile(
        jidx.reshape(N_GATHER, GCH // 16, 16).transpose(2, 0, 1).reshape(16, -1),
        (8, 1))
    # weighted one-hot rhs, precomputed: [p, win, k, (c, a)]
    wone_flat = np.zeros((N_SLOT, 4, WIN), dtype=np.float32)
    wone_flat[np.arange(N_SLOT), :, slot_rank] = slot_coef
    wone_h = np.ascontiguousarray(
        wone_flat.reshape(N_WIN, K_CH, 128, 4 * WIN).transpose(2, 0, 1, 3)
    ).astype(ml_dtypes.bfloat16)
    # counts replicated over 3 vec channels: [w, c, a-rank]
    col_of = (atom_win * WIN + atom_rank).astype(np.int64)
    cnts_col = np.zeros(N_LOC, dtype=np.float32)
    cnts_col[col_of] = counts
    cnt3_h = np.broadcast_to(
        cnts_col.reshape(N_WIN, 1, WIN), (N_WIN, 3, WIN)).reshape(1, -1)
    return (gidx_h, wone_h,
            np.ascontiguousarray(cnt3_h).astype(ml_dtypes.bfloat16), col_of)


def _windows_needed(a_loc):
    counts = np.bincount(a_loc, minlength=ATOMS_PER_CORE)
    w = acc = na = 0
    for atom in range(ATOMS_PER_CORE):
        c = int(counts[atom])
        if acc + c > SLOTS_PER_WIN or na == WIN:
            w += 1
            acc = na = 0
        acc += c
        na += 1
    return w + 1


def _set_n_win(nw):
    g = globals()
    g["N_WIN"] = nw
    g["N_LOC"] = nw * WIN
    g["N_SLOT"] = nw * SLOTS_PER_WIN
    g["N_CHUNK"] = g["N_SLOT"] // 128
    g["N_GATHER"] = g["N_SLOT"] // GCH
    g["N_GRP"] = nw // GRP


def kernel(atomic_embedding, pairlist, f_ij_cutoff, r_ij, W, b):
    atomic_embedding = np.asarray(atomic_embedding, dtype=np.float32)
    pairlist = np.asarray(pairlist)
    f_ij = np.asarray(f_ij_cutoff, dtype=np.float32).reshape(-1)
    r_ij = np.asarray(r_ij, dtype=np.float32)
    W = np.asarray(W, dtype=np.float32)
    b = np.asarray(b, dtype=np.float32)

    u = r_ij / np.linalg.norm(r_ij, axis=1, keepdims=True)
    coef4 = np.concatenate([f_ij[:, None], f_ij[:, None] * u], axis=1)  # [P,4]

    idx_i = np.asarray(pairlist[0], dtype=np.int64)
    idx_j = np.asarray(pairlist[1], dtype=np.int64)
    order = np.argsort(idx_i, kind="stable")
    idx_i_s, idx_j_s, coef_s = idx_i[order], idx_j[order], coef4[order]

    table = atomic_embedding.astype(ml_dtypes.bfloat16)
    wt_h = np.ascontiguousarray(W.T).astype(ml_dtypes.bfloat16)
    b_h = b.reshape(1, F).astype(ml_dtypes.bfloat16)

    bounds = np.searchsorted(idx_i_s, np.arange(0, N_ATOMS + 1, ATOMS_PER_CORE))
    need = max(_windows_needed(idx_i_s[bounds[c]:bounds[c + 1]] - c * ATOMS_PER_CORE)
               for c in range(N_CORES))
    # round up: multiple of GRP (phase-2 groups) and of 2 (1024-idx gathers)
    nw = -(-max(need, 16) // GRP) * GRP
    if nw != N_WIN:
        _cache.pop("nc", None)
    _set_n_win(nw)
    in_maps = []
    colmaps = []
    for c in range(N_CORES):
        lo, hi = bounds[c], bounds[c + 1]
        gidx_h, wone_h, cnt3_h, col_of = _prep_core(
            idx_i_s[lo:hi], idx_j_s[lo:hi], coef_s[lo:hi], c * ATOMS_PER_CORE)
        in_maps.append({
            "table": table, "gidx": gidx_h,
            "wone": wone_h.reshape(128, N_WIN, K_CH, 128),
            "wt": wt_h, "bvec": b_h, "cnt3": cnt3_h,
        })
        colmaps.append(col_of)

    if "nc" not in _cache:
        _cache["nc"] = _build_graph()
    res = run_bass_kernel_spmd(_cache["nc"], in_maps, core_ids=list(range(N_CORES)))

    out_full = np.empty((N_ATOMS, 2 * F), dtype=np.float32)
    for c in range(N_CORES):
        ov = res.results[c]["out_v"]                      # [128, N_LOC] f32
        orad = np.asarray(res.results[c]["out_r"]).astype(np.float32)
        n = ATOMS_PER_CORE
        out_full[c * n:(c + 1) * n, 0:F] = ov[:, colmaps[c]].T
        out_full[c * n:(c + 1) * n, F:] = orad[:, colmaps[c]].T
    return out_full
